# revision 63
# baseline (speedup 1.0000x reference)
"""Linear attention (B=2, L=4096, DM=1024, H=16) on 8 trn2 NeuronCores.

~115.3us vs the 147.3us two-AllReduce bf16 predecessor.

Sharding: rows (B*L) split 8 ways; the only cross-core term is S = K^T Q
per (batch, head), exchanged once as a 256KB bf16 buffer covering both
batches. The collective cost model is dominated by a 15us fixed overhead
(x1.875 for AllReduce), so ONE ReduceScatter+AllGather pair (15.8+21.6us,
no factor) beats one AllReduce (40.4) and crushes two batch-split ARs
(68.5 serialized).

Precision plan (rel-err gate 2e-2; measures 1.675e-2 on device):
 - q/k projections run in fp8 e4m3 with DoubleRow perf mode (0.5 cyc/row,
   2x bf16). Host applies balanced scaling x*a, W/a with a =
   sqrt(std(W)/std(x)) so both operands sit at the same std and W clears
   the e4m3 denormal floor.
 - v and out projections stay bf16: their fp8 error hits the output
   undamped (measured 4.3e-2 / 2.7e-2 -- fails the gate).
 - q/k features are written as fp8 (l-chunk pairs packed side by side in
   one tile) so the S partials also run DoubleRow; the S error is damped
   by the L=4096 averaging. Denominators come from the same fp8 features
   (a consistent normalization, ~0.25% effect).

Schedule (engine queues are in-order; the tile scheduler dispatches
greedily by readiness, so cross-engine contention is managed by engine
CHOICE and wait-hints, not just emission order):
 - the fp8 mega-tensor [Wq|qT|Wk|kT] (4MB = the pre-collective DMA
   floor, ~12us at 360GB/s) is laid out host-side as the exact SBUF
   image and shipped as contiguous SECTIONS in projection order (Wq,
   then per-l-tile slices of qT-batch0, Wk, kT-batch0, then the batch-1
   halves): the first projection unit closes ~8us in and the feature
   chain overlaps the rest of the load. bf16 warmup matmuls from t~0.4
   pre-age the PE clock ramp (13.4us of 0.65GHz after any long idle).
 - bias rides in host-sent fp8 broadcast rows carrying b+1, applied by an
   [I|0] identity-pair DoubleRow matmul (256cyc) emitted last per group.
 - feature copyout is TWO ops per l-tile: Act exp(psum - 1) and one DVE
   scalar_tensor_tensor computing max(min(exp,1), psum) -- exploiting
   exp's monotonicity (min(exp(x),1) == exp(min(x,0))) and elu(x)+1 ==
   max(x+1, exp(min(x,0))), with the +1 pre-added by the bias row. The
   16x1.19us DVE chain is the pre-collective throughput floor; GPSIMD
   cannot read PSUM, so no third engine can share it.
 - S partials run AFTER all four projection batches (in-order PE would
   otherwise stall the projection supply on the feature stts); their
   extraction is one strided copy per partition-half (Act for batch 0,
   DVE for batch 1), then cc_in ships per-batch halves on two DGE paths.
 - the RS+AG window holds the bf16 vT projection (Act-fused per-partition
   bv), denominators (wait-hinted off the stt chain), reciprocal
   broadcasts (PE selector matmuls), then f32 warmup matmuls (853ns each)
   that keep the clock ramp hot until the gathered S lands; the window is
   collective-bound, so mid-window PE gaps are free.
 - tail: attn quadrant matmuls (S^T @ vT per head-pair), one [128,1024]
   DVE reciprocal-multiply per psum pair-tile (attnT/rb are packed per
   t-pair), then the TRANSPOSED out projection (outT = Wo^T attnT, bias
   bo per-partition fused into the Act copyout, stores straight from
   f32 staging tiles; host transposes [DM, ROWS] back). The final group
   tapers [384|128] across fresh psum banks so the drain is one short
   copyout+store deep.
"""
import sys

sys.path.insert(0, "/opt/trn_rl_repo")
import numpy as np
import ml_dtypes

B, L, DM, H = 2, 4096, 1024, 16
D = DM // H  # 64
N_CORES = 8
ROWS = B * L // N_CORES  # 1024 rows per core
RPB = ROWS // B  # 512 rows per batch per core
NT = ROWS // 128  # 8 l-tiles per core (4 per batch)
KC = DM // 128  # 8 contraction chunks
KCP = KC // 2  # 4 fp8 DoubleRow chunk-pairs

_CACHE = {}


def _build():
    import concourse.bass as bass
    import concourse.mybir as mybir
    import concourse.tile as tile
    from concourse import bacc
    from concourse.masks import make_identity

    dt = mybir.dt
    f32, bf16, f8 = dt.float32, dt.bfloat16, dt.float8e4
    AFT = mybir.ActivationFunctionType
    DR = mybir.MatmulPerfMode.DoubleRow

    nc = bacc.Bacc("TRN2", target_bir_lowering=False, debug=False,
                   num_devices=N_CORES)

    W8 = 4 * 1024   # fp8 elements per kc-block: [Wq | qTa | qTb | Wk | kTa | kTb]
    W16 = 3 * 1024  # bf16 elements per kc-block: [Wv | vT | Wo]
    # both mega-tensors arrive as the exact SBUF image [128, KC*W]: loads
    # are plain contiguous column-range copies, and each SECTION (one
    # weight matrix or one batch-half of x) can be shipped separately in
    # dependency order -- the first projection unit closes ~5us in and the
    # feature chain overlaps the rest of the load
    big8_d = nc.dram_tensor("big8", [128, KC * W8], f8,
                            kind="ExternalInput").ap()
    big16_d = nc.dram_tensor("big16", [128, KC * W16], bf16,
                             kind="ExternalInput").ap()
    bq_d = nc.dram_tensor("bqbc", [128, 1536], f8, kind="ExternalInput").ap()
    bk_d = nc.dram_tensor("bkbc", [128, 1536], f8, kind="ExternalInput").ap()
    id8_d = nc.dram_tensor("id8", [128, 256], f8, kind="ExternalInput").ap()
    E_d = nc.dram_tensor("Econst", [16, DM], bf16, kind="ExternalInput").ap()
    bvT_d = nc.dram_tensor("bvT", [128, KC], f32, kind="ExternalInput").ap()
    boT_d = nc.dram_tensor("boT", [128, KC], f32, kind="ExternalInput").ap()
    out_d = nc.dram_tensor("out", [DM, ROWS], f32, kind="ExternalOutput").ap()

    with tile.TileContext(nc) as tc:
        with (
            tc.tile_pool(name="xt", bufs=1) as xt_pool,
            tc.tile_pool(name="act", bufs=1) as act_pool,
            tc.tile_pool(name="tmp", bufs=3) as tmp_pool,
            tc.tile_pool(name="small", bufs=1) as small_pool,
            tc.tile_pool(name="ps", bufs=4, space="PSUM") as ps_pool,
            tc.tile_pool(name="dram", bufs=1, space="DRAM") as dram_pool,
        ):
            ident = small_pool.tile([128, 128], f32, tag="ident", name="ident")
            make_identity(nc, ident[:])
            neg1 = small_pool.tile([128, 1], f32, tag="neg1", name="neg1")
            nc.vector.memset(neg1[:], -1.0)
            wexp = small_pool.tile([128, 1], f32, tag="wexp", name="wexp")
            nc.scalar.activation(wexp[:], neg1[:], AFT.Exp)

            # id8/bias_q ahead of big8 (the q-batch0 bias matmul that
            # stops the first psum group needs them); big8 in 8 kc-splits
            # so the arrival-paced matmuls overlap the (pstate-low) ramp;
            # everything else behind it
            id8 = small_pool.tile([128, 256], f8, tag="id8", name="id8")
            nc.gpsimd.dma_start(id8[:], id8_d)
            bias_q = small_pool.tile([128, 1536], f8, tag="bq", name="bq")
            nc.gpsimd.dma_start(bias_q[:], bq_d)

            # big8 sections (fp8 element offsets in the flat image):
            # Wq 0:8K, qTa 8K:12K, qTb 12K:16K, Wk 16K:24K, kTa 24K:28K,
            # kTb 28K:32K -- shipped in projection order so the feature
            # chain starts as soon as [Wq|qTa] lands
            big8t = xt_pool.tile([128, KC * W8], f8, tag="b8", name="b8")
            # batch-0 x data per-m (1KB sections) so l-tile m0's unit
            # closes the moment [Wq | qm0] lands; batch 1 as whole halves
            splits = [(0, 8192)]
            splits += [(8192 + m * 1024, 9216 + m * 1024) for m in range(4)]
            splits += [(16384, 24576)]
            splits += [(24576 + m * 1024, 25600 + m * 1024)
                       for m in range(4)]
            splits += [(12288, 16384), (28672, 32768)]
            for lo, hi in splits:
                nc.sync.dma_start(big8t[:, lo:hi], big8_d[:, lo:hi])

            def b8sec(base, width):
                return big8t[:, base:base + KC * width].rearrange(
                    "p (kc c) -> p kc c", kc=KC)
            wq_v = b8sec(0, 1024)
            wk_v = b8sec(16384, 1024)
            # per-m views: batch 0 -> its own 1KB section; batch 1 -> slice
            # of the half-section
            qm_v = ([b8sec(8192 + m * 1024, 128) for m in range(4)] +
                    [b8sec(12288, 512) for _ in range(4)])
            km_v = ([b8sec(24576 + m * 1024, 128) for m in range(4)] +
                    [b8sec(28672, 512) for _ in range(4)])

            bias_k = small_pool.tile([128, 1536], f8, tag="bk", name="bk")
            nc.gpsimd.dma_start(bias_k[:], bk_d)
            Et = small_pool.tile([16, DM], bf16, tag="E", name="E")
            nc.gpsimd.dma_start(Et[:], E_d)
            bvT = small_pool.tile([128, KC], f32, tag="bvT", name="bvT")
            nc.gpsimd.dma_start(bvT[:], bvT_d)
            boT = small_pool.tile([128, KC], f32, tag="boT", name="boT")
            nc.gpsimd.dma_start(boT[:], boT_d)
            # big16 sections: Wv 0:8K, vT 8K:16K, Wo 16K:24K (bf16 elems)
            big16t = xt_pool.tile([128, KC * W16], bf16, tag="b16",
                                  name="b16")
            nc.sync.dma_start(big16t[:, 0:16384], big16_d[:, 0:16384])
            nc.sync.dma_start(big16t[:, 16384:24576],
                              big16_d[:, 16384:24576])

            # start-warmers: begin the PE busy-run at ~0.5us so the
            # time-based clock ramp exits its low phase before the
            # post-load crunch
            w0 = small_pool.tile([1, 512], bf16, tag="w0", name="w0")
            nc.vector.memset(w0[:], 1.0)
            wps0 = ps_pool.tile([128, 1024], f32, tag="pp2", name="warm0")
            for i in range(7):
                nc.tensor.matmul(wps0[:, 0:512], w0[0:1, 0:128],
                                 w0[0:1, 0:512],
                                 start=(i == 0), stop=(i == 6))

            # fp8 feature tiles packed [m0-h0 | m0-h1 | m1-h0 | m1-h1] per
            # l-tile pair: one [128,1024] copyout per l-tile, and the S
            # partials contract the l-chunk pair in one DoubleRow matmul
            # (mm-stride 1024 within the tile)
            qp = [act_pool.tile([128, 2048], f8, tag=f"q{mp}",
                                name=f"q{mp}") for mp in range(NT // 2)]
            kp = [act_pool.tile([128, 2048], f8, tag=f"k{mp}",
                                name=f"k{mp}") for mp in range(NT // 2)]

            def pair2(ap):
                return ap.rearrange("p (two c) -> p two c", two=2)

            def proj_batch(xvs, wv, bias_t, outs, mh):
                """One batch of one projection: 4 double-bank psum tiles
                ([128,1024], one per m-tile), fp8 DoubleRow, m-major so
                each l-tile's copyout chain starts the moment its own
                matmuls finish; bias last via [I|0] pair matmul."""
                ms = [mh * 4 + i for i in range(4)]
                psums = {m: ps_pool.tile([128, 1024], f32, tag="pp2",
                                         name="pp2") for m in ms}

                def mm(c, n, m):
                    xv = xvs[m]
                    ml = (m % 4) if xv.shape[2] > 128 else 0
                    nc.tensor.matmul(
                        psums[m][:, n * 512:(n + 1) * 512],
                        xv[:, 2 * c:2 * c + 2, ml * 128:(ml + 1) * 128],
                        wv[:, 2 * c:2 * c + 2, n * 512:(n + 1) * 512],
                        start=(c == 0), stop=False, perf_mode=DR)

                def bias_mm(n, m):
                    nc.tensor.matmul(
                        psums[m][:, n * 512:(n + 1) * 512], pair2(id8[:]),
                        pair2(bias_t[:, n * 512:n * 512 + 1024]),
                        start=False, stop=True, perf_mode=DR)

                # The psum holds x+1 (the host bias rows carry +1), so
                # elu(x)+1 = max(x+1, exp(min(x,0))) = max(min(exp(x),1), x+1)
                # costs just TWO chained ops per l-tile: Act exp(in - 1) and
                # one DVE stt doing the min+max. (exp is monotonic, so
                # min(exp(x),1) == exp(min(x,0)); x ~ N(0,1) cannot
                # overflow exp in f32.)
                def copyout(m):
                    ex = tmp_pool.tile([128, 1024], f32, tag="ex",
                                       name="ex", bufs=4)
                    nc.scalar.activation(ex[:], psums[m][:], AFT.Exp,
                                         bias=neg1[:, 0:1])
                    # all stts on DVE: they read PSUM, which GPSIMD
                    # cannot access on real hardware
                    nc.vector.scalar_tensor_tensor(
                        outs[m // 2][:, (m % 2) * 1024:(m % 2) * 1024 + 1024],
                        ex[:], 1.0, psums[m][:],
                        op0=mybir.AluOpType.min,
                        op1=mybir.AluOpType.max)

                for m in ms:
                    for n in range(2):
                        for c in range(KCP):
                            mm(c, n, m)
                        bias_mm(n, m)
                    copyout(m)

            def s_partial(b):
                """S partial for batch b, head-pair blocks as before but each
                matmul contracts an l-chunk PAIR via DoubleRow."""
                Sp = ps_pool.tile([128, 1024], f32, tag="pp2", name="S_ps")
                S_ps = [Sp[:, 0:512], Sp[:, 512:1024]]
                for g in range(2):
                    for i in range(4):
                        c0 = g * 512 + i * 128
                        for lcp in range(2):
                            mp = b * 2 + lcp
                            nc.tensor.matmul(
                                S_ps[g][:, i * 128:(i + 1) * 128],
                                pair2(kp[mp][:])[:, :, c0:c0 + 128],
                                pair2(qp[mp][:])[:, :, c0:c0 + 128],
                                start=(lcp == 0), stop=(lcp == 1),
                                perf_mode=DR)
                return S_ps

            ccst = small_pool.tile([128, 1024], bf16, tag="ccst", name="ccst")

            def extract(b, S_ps):
                # diagonal quarters -> ccst cols [b*512 : (b+1)*512];
                # split across Act and DVE so the 4 copies drain in ~2 slots
                for g in range(2):
                    for j in range(2):
                        src = S_ps[g][j * 64:(j + 1) * 64, :].rearrange(
                            "p (i c) -> p i c", i=4)[:, :,
                                                     j * 64:(j + 1) * 64]
                        dst = ccst[j * 64:(j + 1) * 64,
                                   b * 512 + g * 256:
                                   b * 512 + (g + 1) * 256].rearrange(
                            "p (i d) -> p i d", i=4)
                        if b:
                            nc.vector.tensor_copy(dst, src)
                        else:
                            nc.scalar.activation(dst, src, AFT.Copy)

            # ---- batch 0 (paced against the mega-load), batch 1; S
            # partials LAST so the in-order PE queue never stalls the
            # projection supply on the feature-chain stts ----
            proj_batch(qm_v, wq_v, bias_q, qp, 0)
            proj_batch(km_v, wk_v, bias_k, kp, 0)
            proj_batch(qm_v, wq_v, bias_q, qp, 1)
            proj_batch(km_v, wk_v, bias_k, kp, 1)
            S0 = s_partial(0)
            extract(0, S0)
            S1 = s_partial(1)
            extract(1, S1)

            cc_in = dram_pool.tile([128, 1024], bf16, tag="ccin", name="ccin")
            cc_rs = dram_pool.tile([16, 1024], bf16, tag="ccrs", name="ccrs")
            cc_out = dram_pool.tile([128, 1024], bf16, tag="ccout",
                                    name="ccout")
            # batch-0 half ships as soon as its extract lands; only the
            # batch-1 half is on the critical path to the collective
            nc.gpsimd.dma_start(cc_in[:, 0:512], ccst[:, 0:512])
            nc.sync.dma_start(cc_in[0:64, 512:1024], ccst[0:64, 512:1024])
            nc.sync.dma_start(cc_in[64:128, 512:1024],
                              ccst[64:128, 512:1024])
            # ReduceScatter + AllGather instead of AllReduce: the cost
            # model charges AllReduce 1.875x its size-based time, while
            # RS+AG pay the (dominant) fixed overhead twice but no factor
            # -- net ~3us cheaper for this 256KB payload
            nc.gpsimd.collective_compute(
                "ReduceScatter", mybir.AluOpType.add,
                replica_groups=[list(range(N_CORES))],
                ins=[cc_in[:].opt()], outs=[cc_rs[:].opt()])
            nc.gpsimd.collective_compute(
                "AllGather", mybir.AluOpType.bypass,
                replica_groups=[list(range(N_CORES))],
                ins=[cc_rs[:].opt()], outs=[cc_out[:].opt()])
            # ccJ reload: halves on two independent DMA paths (SP HWDGE
            # and Pool SWDGE) so they land in parallel; the j=0 attn
            # matmuls only need rows 0:64
            ccJ = small_pool.tile([128, 1024], bf16, tag="ccJ", name="ccJ")
            nc.sync.dma_start(ccJ[0:64, :], cc_out[0:64, :])
            nc.gpsimd.dma_start(ccJ[64:128, :], cc_out[64:128, :])
            ccJs = [ccJ, ccJ]

            # ---- denominators on the Pool engine (free once the extracts
            # are done); they only need to be ready for dent_half during
            # the AR window ----
            dens = []
            for m in range(NT):
                den = tmp_pool.tile([128, 16], f32, tag="den", name="den",
                                    bufs=NT)
                for half in range(2):
                    prod = tmp_pool.tile([128, 512], bf16, tag="prod",
                                         name="prod")
                    sl = slice((m % 2) * 1024 + half * 512,
                               (m % 2) * 1024 + half * 512 + 512)
                    # muls on Pool (SBUF-only, legal there; Pool is idle
                    # during the feature chain); the X-axis reduce is
                    # DVE-only and wait-hinted past the feature chain (only
                    # needed by dent_half inside the AR window)
                    with tc.tile_wait_until(0.043):
                        nc.vector.tensor_mul(prod[:], qp[m // 2][:, sl],
                                             kp[m // 2][:, sl])
                        nc.vector.reduce_sum(
                            den[:, half * 8:(half + 1) * 8],
                            prod[:].rearrange("p (h d) -> p h d", h=8),
                            axis=mybir.AxisListType.X)
                dens.append(den)

            # ---- vT projection + recip broadcasts fill the AR window ----
            recipT = small_pool.tile([16, ROWS], f32, tag="recipT",
                                     name="recipT")
            recipT_r = small_pool.tile([16, ROWS], bf16, tag="recipTr",
                                       name="recipTr")
            # rb/attnT packed per t-pair [t-even cols 0:1024 | t-odd
            # 1024:2048] so one [128,1024] DVE mul covers a whole psum
            # pair-tile in the attn phase
            rbp = [act_pool.tile([128, 2 * ROWS], bf16, tag=f"rb{tp}",
                                 name=f"rb{tp}") for tp in range(KC // 2)]

            def dent_half(b):
                dentt = ps_pool.tile([128, 1024], f32, tag="pp2",
                                     name="dent")
                for i, m in enumerate(range(b * 4, b * 4 + 4)):
                    dent = dentt[0:16, i * 128:(i + 1) * 128]
                    nc.tensor.transpose(dent, dens[m][:], ident[:])
                    nc.vector.tensor_scalar_add(
                        recipT[:, m * 128:(m + 1) * 128], dent, 1e-6)
                sl = slice(b * 512, (b + 1) * 512)
                nc.vector.reciprocal(recipT[:, sl], recipT[:, sl])
                nc.vector.tensor_copy(recipT_r[:, sl], recipT[:, sl])

            def rb_half(b):
                for u in range(KC // 2):
                    psr = ps_pool.tile([128, 1024], f32, tag="pp2",
                                       name="psr")
                    for half in range(2):
                        t = 2 * u + half
                        nc.tensor.matmul(psr[:, half * 512:(half + 1) * 512],
                                         Et[:, t * 128:(t + 1) * 128],
                                         recipT_r[:, b * 512:(b + 1) * 512],
                                         start=True, stop=True)
                    for half in range(2):
                        t = 2 * u + half
                        nc.scalar.activation(
                            rbp[u][:, half * 1024 + b * 512:
                                   half * 1024 + (b + 1) * 512],
                            psr[:, half * 512:(half + 1) * 512], AFT.Copy)

            vTs = [act_pool.tile([128, ROWS], bf16, tag=f"vt{t}",
                                 name=f"vt{t}")
                   for t in range(KC)]
            for t in range(KC):
                ps2t = ps_pool.tile([128, 1024], f32, tag="pp2", name="pp")
                ps2 = [ps2t[:, 0:512], ps2t[:, 512:1024]]
                for kc in range(KC):
                    for n in range(2):
                        nc.tensor.matmul(
                            ps2[n][:],
                            big16t[:, kc * 1024 + t * 128:
                                   kc * 1024 + (t + 1) * 128],
                            big16t[:, 8192 + kc * 1024 + n * 512:
                                   8192 + kc * 1024 + (n + 1) * 512],
                            start=(kc == 0), stop=(kc == KC - 1))
                for n in range(2):
                    nc.scalar.activation(
                        vTs[t][:, n * 512:(n + 1) * 512], ps2[n],
                        AFT.Identity, bias=bvT[:, t:t + 1])

            # ---- tail: attn for both batches, transposed out projection ----
            attnT = [act_pool.tile([128, 2 * ROWS], bf16, tag=f"at{tp}",
                                    name=f"attnT{tp}")
                     for tp in range(KC // 2)]

            def attn_half(b):
                # all 16 matmuls back-to-back (no interleaved consumers):
                # any SEQ stall between singleton matmuls resets the PE
                # p-state ramp and the whole phase runs at 0.65 GHz
                pts = [ps_pool.tile([128, 1024], f32, tag="pp2", name="pa")
                       for _ in range(KC // 2)]
                pss = [pts[t // 2][:, (t % 2) * 512:(t % 2) * 512 + 512]
                       for t in range(KC)]
                # matmuls per t as before; the mul runs once per pair
                for t in range(KC):
                    for j in range(2):
                        col = b * 512 + (t // 4) * 256 + (t % 4) * 64
                        nc.tensor.matmul(
                            pss[t][j * 64:(j + 1) * 64, :],
                            ccJs[j][j * 64:(j + 1) * 64, col:col + 64],
                            vTs[t][j * 64:(j + 1) * 64,
                                   b * RPB:(b + 1) * RPB],
                            start=True, stop=True)
                for tp in range(KC // 2):
                    def pv(ap):
                        return ap.rearrange("p (two c) -> p two c",
                                            two=2)[:, :,
                                                   b * RPB:(b + 1) * RPB]
                    nc.vector.tensor_mul(
                        pv(attnT[tp][:]), pts[tp][:].rearrange(
                            "p (two c) -> p two c", two=2),
                        pv(rbp[tp][:]))

            def out_half(b, taper=False):
                pot = None
                for t in range(KC):
                    if t % 2 == 0:
                        pot = ps_pool.tile([128, 1024], f32, tag="pp2",
                                           name="po")
                    ps = pot[:, (t % 2) * 512:(t % 2) * 512 + 512]
                    c0 = b * 512
                    if taper and t == KC - 1:
                        # taper in a FRESH psum tile: both sub-groups land
                        # on banks with no pending zero-region from this
                        # tile generation, so neither waits on a copyout
                        tap = ps_pool.tile([128, 1024], f32, tag="pp2",
                                           name="tap")
                        ps = tap[:, 0:512]
                        ot = tmp_pool.tile([128, 512], f32, tag="ex",
                                           name="ot", bufs=4)
                        for kc in range(KC):
                            nc.tensor.matmul(
                                ps[:, 0:384],
                                big16t[:, 16384 + kc * 1024 + t * 128:
                                       16384 + kc * 1024 + (t + 1) * 128],
                                attnT[kc // 2][:, (kc % 2) * 1024 +
                                               c0:(kc % 2) * 1024 + c0 + 384],
                                start=(kc == 0), stop=(kc == KC - 1))
                        nc.scalar.activation(ot[:, 0:384], ps[:, 0:384],
                                             AFT.Identity,
                                             bias=boT[:, t:t + 1])
                        nc.sync.dma_start(
                            out_d[t * 128:(t + 1) * 128, c0:c0 + 384],
                            ot[:, 0:384])
                        # sliver in its OWN fresh psum tile: PSUM deps
                        # are tile-granular, so sharing the 384-group's
                        # tile would serialize the sliver matmuls behind
                        # that group's Act copyout
                        tap2 = ps_pool.tile([128, 1024], f32, tag="pp2",
                                            name="tap2")
                        slv = tap2[:, 0:128]
                        for kc in range(KC):
                            nc.tensor.matmul(
                                slv,
                                big16t[:, 16384 + kc * 1024 + t * 128:
                                       16384 + kc * 1024 + (t + 1) * 128],
                                attnT[kc // 2][:, (kc % 2) * 1024 + c0 +
                                               384:(kc % 2) * 1024 + c0 + 512],
                                start=(kc == 0), stop=(kc == KC - 1))
                        nc.vector.tensor_scalar(
                            ot[:, 384:512], slv,
                            boT[:, t:t + 1], None,
                            op0=mybir.AluOpType.add)
                        nc.sync.dma_start(
                            out_d[t * 128:(t + 1) * 128,
                                  c0 + 384:c0 + 512],
                            ot[:, 384:512])
                        continue
                    for kc in range(KC):
                        nc.tensor.matmul(
                            ps[:],
                            big16t[:, 16384 + kc * 1024 + t * 128:
                                   16384 + kc * 1024 + (t + 1) * 128],
                            attnT[kc // 2][:, (kc % 2) * 1024 +
                                           c0:(kc % 2) * 1024 + c0 + 512],
                            start=(kc == 0), stop=(kc == KC - 1))
                    ot = tmp_pool.tile([128, 512], f32,
                                       tag=("mn" if t % 2 else "ex"),
                                       name="ot", bufs=4)
                    # alternate copyout engines so the drain isn't
                    # serialized behind one engine's queue
                    if t % 2:
                        nc.scalar.activation(ot[:], ps[:], AFT.Identity,
                                             bias=boT[:, t:t + 1])
                    else:
                        nc.vector.tensor_scalar(
                            ot[:], ps[:], boT[:, t:t + 1], None,
                            op0=mybir.AluOpType.add)
                    nc.sync.dma_start(
                        out_d[t * 128:(t + 1) * 128, c0:c0 + 512], ot[:])

            dent_half(0)
            dent_half(1)
            rb_half(0)
            rb_half(1)

            # PE p-state warmers: the sim's clock ramp restarts after a
            # long idle (~13.4us of 0.65GHz before full speed), and the AR
            # window leaves the PE idle after vT/rb, which would put the
            # attn phase and the first out groups at low clock. Burn the
            # idle with f32 dummy matmuls (4 cyc/row -> 853ns each at full
            # clock) into a scratch psum: genuinely continuous PE work, no
            # cross-engine pacing chain needed.
            wps = ps_pool.tile([128, 1024], f32, tag="pp2", name="warm")
            NWARM = 8
            for i in range(NWARM):
                nc.tensor.matmul(wps[:, 0:512], recipT[0:16, 0:128],
                                 recipT[0:16, 0:512],
                                 start=(i == 0), stop=(i == NWARM - 1))

            attn_half(0)
            attn_half(1)
            out_half(0)
            out_half(1, taper=True)

    nc.compile()
    return nc


def _get_nc():
    if "nc" not in _CACHE:
        _CACHE["nc"] = _build()
    return _CACHE["nc"]


def _make_econst():
    E = np.zeros((16, DM), np.float32)
    for t in range(KC):
        E[2 * t, t * 128:t * 128 + 64] = 1.0
        E[2 * t + 1, t * 128 + 64:(t + 1) * 128] = 1.0
    return E


def kernel(query, key, value, Wq, bq, Wk, bk, Wv, bv, Wo, bo, **kw):
    from concourse.bass_utils import run_bass_kernel_spmd

    nc = _get_nc()
    F8 = ml_dtypes.float8_e4m3fn
    BF = ml_dtypes.bfloat16
    query = np.asarray(query, dtype=np.float32)
    key = np.asarray(key, dtype=np.float32)
    value = np.asarray(value, dtype=np.float32)
    Wq = np.asarray(Wq, np.float32)
    Wk = np.asarray(Wk, np.float32)
    aq = float(np.sqrt(Wq.std() / max(query.std(), 1e-30)))
    ak = float(np.sqrt(Wk.std() / max(key.std(), 1e-30)))
    Wq8 = (Wq / aq).astype(F8)
    Wk8 = (Wk / ak).astype(F8)
    Wv16 = np.asarray(Wv, np.float32).astype(BF)
    Wo16 = np.asarray(Wo, np.float32).astype(BF)

    def bias_bc(b):
        # +1 rides in the bias so the projection psum holds x+1 directly
        # (the ELU copyout identity needs it); fp8 represents 1.0 exactly
        t = np.zeros((128, 1536), np.float32)
        t[:, :1024] = np.asarray(b, np.float32).reshape(1, DM) + 1.0
        return t.astype(F8)

    bqbc = bias_bc(bq)
    bkbc = bias_bc(bk)
    id8 = np.zeros((128, 256), np.float32)
    id8[:, :128] = np.eye(128)
    id8 = id8.astype(F8)
    econst = _make_econst().astype(BF)
    bvT = np.ascontiguousarray(
        np.asarray(bv, np.float32).reshape(KC, 128).T)
    boT = np.ascontiguousarray(
        np.asarray(bo, np.float32).reshape(KC, 128).T)

    def sec(x):
        # [DM, C] -> the SBUF image section [128, KC*C] (kc-major blocks)
        C = x.shape[1]
        return x.reshape(KC, 128, C).transpose(1, 0, 2).reshape(128, KC * C)

    wq_s = sec(Wq8)
    wk_s = sec(Wk8)
    wv_s = sec(Wv16)
    wo_s = sec(Wo16)
    in_maps = []
    for c in range(N_CORES):
        sl = slice(c * RPB, (c + 1) * RPB)
        qT = (np.concatenate([query[b, sl] for b in range(B)], 0).T
              * aq).astype(F8)
        kT = (np.concatenate([key[b, sl] for b in range(B)], 0).T
              * ak).astype(F8)
        vT = np.concatenate([value[b, sl] for b in range(B)], 0).T.astype(BF)
        big8 = np.concatenate(
            [wq_s] + [sec(qT[:, m * 128:(m + 1) * 128]) for m in range(4)]
            + [sec(qT[:, 512:1024]), wk_s]
            + [sec(kT[:, m * 128:(m + 1) * 128]) for m in range(4)]
            + [sec(kT[:, 512:1024])], axis=1)
        big16 = np.concatenate([wv_s, sec(vT), wo_s], axis=1)
        m = {
            "big8": np.ascontiguousarray(big8),
            "big16": np.ascontiguousarray(big16),
            "bqbc": bqbc, "bkbc": bkbc, "id8": id8,
            "Econst": econst, "bvT": bvT, "boT": boT,
        }
        in_maps.append(m)

    res = run_bass_kernel_spmd(nc, in_maps, list(range(N_CORES)), **kw)
    out = np.empty((B, L, DM), np.float32)
    for c in range(N_CORES):
        o = np.asarray(res.results[c]["out"]).astype(np.float32)
        for b in range(B):
            out[b, c * RPB:(c + 1) * RPB] = o[:, b * RPB:(b + 1) * RPB].T
    if kw:
        return out, res
    return out
